# revision 5
# baseline (speedup 1.0000x reference)
"""Trainium2 Bass kernel for nn_DenseAttentionOneHead — v3: fp8 S phase.

out_b = X_b (W^T (X_b^T X_b)).  Column-split 8 ways (4 slices x 2 batches),
collective-free.  Per core (slice sl of 256 cols, inputs column-rotated so
the slice is cols 0:256):

  S^T_sl = (X^T X[:, sl])^T     fp8 DoubleRow: pairs of n-chunks contracted
                                256 rows/matmul, Nf=512 -> 64 matmuls, 2x rate
  Ssb    = XBAR-transpose(S^T)  DMA crossbar, zero PE cost
  M_sl   = W^T S_sl             fp16, 64 matmuls
  out^T[:, sl] = M_sl^T X^T     fp16, 128 matmuls Nf=512

fp8 cuts the S phase PE time in half (157 TF/s) and the X stream to 4MB,
which leaves HBM room to preload W and X^T slab0 during/after S without
starving the chunk stream (the v2b fp16 attempt showed X at fp16 needs the
whole S-phase bandwidth).  Accuracy: S from fp8 X gives rel_rms ~1.7e-2
(vs 2e-2 gate); W/X stay fp16 in the M/out phases which dominate error
sensitivity.
"""

import numpy as np
import ml_dtypes

import concourse.mybir as mybir
import concourse.tile as tile
from concourse import bacc
from concourse.bass_utils import run_bass_kernel_spmd

F32 = mybir.dt.float32
F16 = mybir.dt.float16
F8 = mybir.dt.float8e4
DR = mybir.MatmulPerfMode.DoubleRow
P = 128
D = 1024
B = 2
N = 4096
NCORES = 8
GROUP = 4            # cores per batch
SL = D // GROUP      # 256-column slice per core
NO = D // P          # 8 tiles along D
NPAIR = N // (2 * P)  # 16 chunk-pairs of 256 rows
WSCALE = 64.0        # host-side W prescale (fp16 subnormal dodge)

_compiled = None


def _build():
    nc = bacc.Bacc(None, target_bir_lowering=False, debug=False, num_devices=NCORES)

    # xp8: fp8 chunk-pairs, row (pair*128+p) = [x8[pair*256+p,:], x8[pair*256+128+p,:]]
    # columns rotated per core (its 256 target columns first); wf row-rotated
    # identically; xt is the plain X^T.
    xp8 = nc.dram_tensor("xp8", [NPAIR * P, 2, D], F8, kind="ExternalInput")
    eye = nc.dram_tensor("eye", [P, P], F16, kind="ExternalInput")
    xt = nc.dram_tensor("xt", [D, N], F16, kind="ExternalInput")
    wf = nc.dram_tensor("wf", [D, D], F16, kind="ExternalInput")
    o_out = nc.dram_tensor("o_out", [SL, N], F16, kind="ExternalOutput")

    with tile.TileContext(nc) as tc:
        with (
            tc.tile_pool(name="big", bufs=1) as big,
            tc.tile_pool(name="xin", bufs=NPAIR) as xin,
            tc.tile_pool(name="stage", bufs=6) as stage,
            tc.tile_pool(name="psum", bufs=8, space="PSUM") as psum,
        ):
            XT = big.tile([P, NO, N], F16, tag="XT")        # X^T [a, n], 8MB
            Wsb = big.tile([P, NO, D], F16, tag="W")        # W   [e, a], 2MB
            STsb = big.tile([P, 2, D], F16, tag="STsb")     # S^T [d', e]
            Ssb = big.tile([P, NO, SL], F16, tag="Ssb")     # S   [e, d']
            Msb = big.tile([P, NO, SL], F16, tag="Msb")     # M   [a, d']
            junk = big.tile([P, P], F16, tag="junk")
            eyesb = big.tile([P, P], F16, tag="eye")

            # HAM warmup: throwaway matmuls during the first-DMA window so the
            # PE clock ramp is underway when pair 0 lands.
            nc.vector.memset(junk[:], 0)
            jacc = psum.tile([P, 512], F32, tag="acc", name="jacc")[:, :P]
            for _ in range(20):
                nc.tensor.matmul(jacc[:], junk[:], junk[:], start=True, stop=True)

            # ---- S^T = (X^T X[:, sl])^T via fp8 DoubleRow: 4 held PSUM
            # accumulators (dt x half), 64 matmuls over 16 chunk-pairs.
            sts = {
                (dt, h): psum.tile([P, 512], F32, tag="acc", name=f"st_{dt}{h}")
                for dt in range(2)
                for h in range(2)
            }
            prime = xin.tile([P, 32], F8, tag="prime")
            nc.sync.dma_start(prime[:, 0:16], xp8[0:P, 0, 0:16])
            nc.scalar.dma_start(prime[:, 16:32], xp8[0:P, 0, 16:32])
            xps = []
            for pr in range(NPAIR):
                xp = xin.tile([P, 2, D], F8, tag="xp")
                xps.append(xp)
                if pr == 0:
                    # quarters on both queues: the first two matmuls only need
                    # cols 0:512 of both chunks, so they start sooner
                    for h in range(2):
                        for i in range(2):
                            qeng = nc.sync if i == 0 else nc.scalar
                            qeng.dma_start(
                                xp[:, i, h * 512:(h + 1) * 512],
                                xp8[0:P, i, h * 512:(h + 1) * 512],
                            )
                else:
                    eng = nc.sync if pr % 2 == 0 else nc.scalar
                    eng.dma_start(xp[:], xp8[pr * P:(pr + 1) * P, :, :])
                for h in range(2):
                    for dt in range(2):
                        nc.tensor.matmul(
                            sts[(dt, h)][:],
                            xp[:, :, dt * P:(dt + 1) * P],
                            xp[:, :, h * 512:(h + 1) * 512],
                            start=(pr == 0),
                            stop=(pr == NPAIR - 1),
                            perf_mode=DR,
                        )

            nc.vector.tensor_copy(eyesb[:, 0:1], xps[1][:, 0, 0:1])
            nc.sync.dma_start(eyesb[:], eye[:])

            # W gated on pair 12's landing (~t=16, when the 4MB fp8 stream is
            # nearly done): early gates make tile schedule the gated issues
            # BETWEEN the pair issues in the engine stream, which stalls the
            # whole pair stream behind the gate (v3 first run: pairs issued at
            # t=34µs, S span 39µs).  Late gates keep the pair issues first.
            # W transfers ~17-23, M consumes from ~27.  slab0 on scalar the
            # same way (needed ~35).
            for wch in range(NO):
                nc.vector.tensor_copy(Wsb[:, wch, 0:1], xps[12][:, 0, 0:1])
            for wch in range(NO):
                nc.sync.dma_start(Wsb[:, wch, :], wf[wch * P:(wch + 1) * P, :])
            # slab0 early on scalar in 512KB quarters (landed ~25, out needs
            # it ~37); quartering keeps completion-semaphore slots recycling
            # fast so the later transposes never wait behind a 2MB transfer.
            for q in range(4):
                nc.vector.tensor_copy(
                    XT[:, 0, q * 256:q * 256 + 1], xps[14][:, 0, 0:1]
                )
            for q in range(4):
                srcq = xt[:, q * 256:(q + 1) * 256]
                nc.scalar.dma_start(
                    XT[:, :, q * 256:(q + 1) * 256],
                    srcq.rearrange("(c p) n -> p c n", p=P),
                )


            # ---- PE transpose S^T -> S (e-major), interleaved with the M
            # ladder.  The XBAR DMA transpose was faster on paper but its
            # completion signalling raced the M reads on some cores (v3e gave
            # a corrupted slice on core 1); PE transposes synchronize through
            # engine semaphores, which are reliable.  Two ping-pong PSUM
            # tiles; drains alternate DVE/ACT.  M runs as two half-ladders
            # (at 0-3 with the transposes, then at 4-7) so everything fits in
            # 8 PSUM banks.
            # interleave the transposes with the first half of the M ladder:
            # the PE stays busy (no HAM downshift) and the transpose drains
            # pipeline behind it.
            # fillers: cover the ~1.2us first-drain latency after S so the
            # PE never idles long enough for a HAM downshift.  They read
            # pair 15's tile so the scheduler cannot hoist them to the
            # warmup region (junk-only fillers got hoisted and did nothing).
            for _ in range(10):
                nc.tensor.matmul(
                    jacc[:], xps[15][:, 0, 0:P], xps[15][:, 0, 0:P],
                    start=True, stop=True,
                )

            tpa = psum.tile([P, P], F16, tag="acc", name="tpa")
            tpb = psum.tile([P, P], F16, tag="acc", name="tpb")
            maccs = [
                psum.tile([P, 512], F32, tag="acc", name=f"macc_{at}")[:, :SL]
                for at in range(4)
            ]
            for ech in range(NO):
                if ech % 2 == 0:
                    # just-in-time S^T drain halves: draining everything up
                    # front left the PE idle ~1.3us at T-start, tripping a
                    # HAM downshift that ran all of T+L1 at half clock
                    p_ = ech // 2
                    h, q2 = p_ // 2, p_ % 2
                    sl_ = slice(q2 * 256, (q2 + 1) * 256)
                    dst = slice(ech * P, (ech + 2) * P)
                    nc.vector.tensor_copy(STsb[:, 0, dst], sts[(0, h)][:, sl_])
                    nc.scalar.copy(STsb[:, 1, dst], sts[(1, h)][:, sl_])
                nc.tensor.transpose(tpa[:], STsb[:, 0, ech * P:(ech + 1) * P], eyesb[:])
                nc.tensor.transpose(tpb[:], STsb[:, 1, ech * P:(ech + 1) * P], eyesb[:])
                nc.vector.tensor_copy(Ssb[:, ech, 0:P], tpa[:])
                nc.scalar.copy(Ssb[:, ech, P:2 * P], tpb[:])
                for at in range(4):
                    nc.tensor.matmul(
                        maccs[at][:],
                        Wsb[:, ech, at * P:(at + 1) * P],
                        Ssb[:, ech, :],
                        start=(ech == 0),
                        stop=(ech == NO - 1),
                    )

            # slabs 1-3 as whole 2MB transfers on sync, gated on the FIRST
            # transposed piece: transfers run ~30-47, overlapping the M phase
            # (whose PE reads touch only Wsb/Ssb, not XT) instead of the out
            # phase — concurrent DMA writes into XT while the out matmuls
            # stream it cost ~20% PE rate in v4b.
            for j in range(1, 4):
                nc.vector.tensor_copy(
                    XT[:, 0, j * 1024:j * 1024 + 1], Ssb[:, 0, 0:1]
                )
            for j in range(1, 4):
                srcx = xt[:, j * 1024:(j + 1) * 1024]
                nc.sync.dma_start(
                    XT[:, :, j * 1024:(j + 1) * 1024],
                    srcx.rearrange("(c p) n -> p c n", p=P),
                )

            # ---- second half of the M ladder (at 4-7; banks freed by the
            # S^T and transpose drains above).
            maccs2 = [
                psum.tile([P, 512], F32, tag="acc", name=f"macc_{at}")[:, :SL]
                for at in range(4, NO)
            ]
            for ech in range(NO):
                for at in range(4, NO):
                    nc.tensor.matmul(
                        maccs2[at - 4][:],
                        Wsb[:, ech, at * P:(at + 1) * P],
                        Ssb[:, ech, :],
                        start=(ech == 0),
                        stop=(ech == NO - 1),
                    )
            for at in range(NO):
                acc = maccs[at] if at < 4 else maccs2[at - 4]
                if at % 2 == 0:
                    nc.vector.tensor_copy(Msb[:, at, :], acc[:])
                else:
                    nc.scalar.copy(Msb[:, at, :], acc[:])

            # ---- out^T[sl, n] = M^T X^T: lhsT = M[a_ch, sl_t] (shared across
            # the n-pair), rhs = XT[a_ch, n-chunk].
            for np_ in range(4):
                oaccs = {
                    (slt, k): psum.tile(
                        [P, 512], F32, tag="acc", name=f"oacc_{np_}_{slt}_{k}"
                    )
                    for slt in range(2)
                    for k in range(2)
                }
                for slt in range(2):
                    for ach in range(NO):
                        for k in range(2):
                            nch = 2 * np_ + k
                            nc.tensor.matmul(
                                oaccs[(slt, k)][:],
                                Msb[:, ach, slt * P:(slt + 1) * P],
                                XT[:, ach, nch * 512:(nch + 1) * 512],
                                start=(ach == 0),
                                stop=(ach == NO - 1),
                            )
                    for k in range(2):
                        nch = 2 * np_ + k
                        ot = stage.tile([P, 512], F16, tag="ot")
                        if np_ < 3 or slt == 0:
                            if slt == 0:
                                nc.vector.tensor_copy(ot[:], oaccs[(slt, k)][:])
                            else:
                                nc.scalar.copy(ot[:], oaccs[(slt, k)][:])
                            weng = nc.sync if k == 0 else nc.scalar
                            weng.dma_start(
                                o_out[slt * P:(slt + 1) * P,
                                      nch * 512:(nch + 1) * 512],
                                ot[:],
                            )
                        else:
                            # final pair: half-pieces on both engines/queues so
                            # the tail drains+writes pipeline
                            for h in range(2):
                                sl_ = slice(h * 256, (h + 1) * 256)
                                if h == 0:
                                    nc.vector.tensor_copy(
                                        ot[:, sl_], oaccs[(slt, k)][:, sl_])
                                else:
                                    nc.scalar.copy(
                                        ot[:, sl_], oaccs[(slt, k)][:, sl_])
                                weng = nc.sync if h == 0 else nc.scalar
                                weng.dma_start(
                                    o_out[slt * P:(slt + 1) * P,
                                          nch * 512 + h * 256:nch * 512 + (h + 1) * 256],
                                    ot[:, sl_],
                                )

    nc.finalize()
    return nc


def _get_compiled():
    global _compiled
    if _compiled is None:
        _compiled = _build()
    return _compiled


def kernel(hidden_states, queries, _trace=False, _trace_cores=None):
    x = np.ascontiguousarray(np.asarray(hidden_states, dtype=np.float32))
    w = np.ascontiguousarray(np.asarray(queries, dtype=np.float32))
    assert x.shape == (B, N, D) and w.shape == (D, D)

    nc = _get_compiled()
    w16 = (w * WSCALE).astype(np.float16)
    eye16 = np.eye(P, dtype=np.float16)
    xt16 = [np.ascontiguousarray(x[b].T.astype(np.float16)) for b in range(B)]
    in_maps = []
    for c in range(NCORES):
        b, s = c // GROUP, c % GROUP
        xrot = np.roll(x[b], -s * SL, axis=1)
        x8 = xrot.astype(ml_dtypes.float8_e4m3fn)
        # pack pairs: row (pair*128+p) = [chunk 2*pair row p, chunk 2*pair+1 row p]
        xp8 = np.ascontiguousarray(
            x8.reshape(NPAIR, 2, P, D).transpose(0, 2, 1, 3).reshape(NPAIR * P, 2, D)
        )
        in_maps.append(
            {
                "xp8": xp8,
                "eye": eye16,
                "xt": xt16[b],
                "wf": np.ascontiguousarray(np.roll(w16, -s * SL, axis=0)),
            }
        )

    res = run_bass_kernel_spmd(
        nc,
        in_maps,
        core_ids=list(range(NCORES)),
        trace=_trace,
        trace_cores=_trace_cores,
    )

    out = np.empty((B, N, D), dtype=np.float32)
    inv = 1.0 / WSCALE
    for c in range(NCORES):
        b, s = c // GROUP, c % GROUP
        ot = res.results[c]["o_out"].astype(np.float32)
        out[b, :, s * SL:(s + 1) * SL] = ot.T * inv

    if _trace:
        kernel.last_result = res
    return out
